# revision 11
# baseline (speedup 1.0000x reference)
"""Low-rank linear attention (causal, elu+1 feature map) on 8 trn2 cores.

Sharding: core = 2*b + h  (batch b in 0..3, sequence half h in 0..1).
Each core computes out[b, h*2048:(h+1)*2048, :].  Second-half cores
recompute the running K^T.V state over their 2048-token prefix on device
(sel input scales the prefix contribution to zero on first-half cores so
one SPMD program serves all 8 cores).

Shapes (hardcoded): B=4, S=4096, D=1024, K=64.  L = S/2 = 2048 tokens
per core, processed in 16 chunks of C=128.
"""

import numpy as np

B, S, D, K = 4, 4096, 1024, 64
L = S // 2          # tokens per core (main), also prefix length
C = 128             # chunk (tokens)
G = 512             # token group for x loads / P1 matmuls
NCHUNK = L // C     # 16
NGRP = L // G       # 4
NDC = D // 128      # 8 contraction chunks

_cache = {}


def _build_nc():
    import concourse.bacc as bacc
    import concourse.tile as tile
    from concourse import mybir

    f32 = mybir.dt.float32
    bf16 = mybir.dt.bfloat16
    AF = mybir.ActivationFunctionType

    nc = bacc.Bacc()

    xtm = nc.declare_dram_parameter("xtm", [D, L], bf16, isOutput=False)
    xtp = nc.declare_dram_parameter("xtp", [D, L], bf16, isOutput=False)
    wcat = nc.declare_dram_parameter("wcat", [128, 2 * D + C], bf16, isOutput=False)
    wot = nc.declare_dram_parameter("wot", [K, D], bf16, isOutput=False)
    sel = nc.declare_dram_parameter("sel", [C, 1], f32, isOutput=False)
    out = nc.declare_dram_parameter("out", [L, D], f32, isOutput=True)

    with tile.TileContext(nc) as tc:
        with (
            tc.tile_pool(name="consts", bufs=1) as consts,
            tc.tile_pool(name="xm", bufs=2) as xm_pool,
            tc.tile_pool(name="xp", bufs=2) as xp_pool,
            tc.tile_pool(name="small", bufs=3) as small,
            tc.tile_pool(name="vko", bufs=2) as vko_pool,
            tc.tile_pool(name="qk", bufs=2) as qk_pool,
            tc.tile_pool(name="tmp", bufs=3) as tmp_pool,
            tc.tile_pool(name="ostage", bufs=2) as ostage_pool,
            tc.tile_pool(name="state_pool", bufs=1, space="PSUM") as state_pool,
            tc.tile_pool(name="p1_ps", bufs=2, space="PSUM") as p1_pool,
            tc.tile_pool(name="p2_ps", bufs=2, space="PSUM") as p2_pool,
            tc.tile_pool(name="atnd_ps", bufs=3, space="PSUM") as atnd_pool,
        ):
            # ---- constants to SBUF (one DMA; early LDW touches teach the
            # PE clock these queues so real matmuls carry <=1 wait) ----
            wcat_sb = consts.tile([128, 2 * D + C], bf16, tag="wcat")
            nc.sync.dma_start(out=wcat_sb, in_=wcat[:, :])
            wqk_sb = [wcat_sb[:, d * 128:(d + 1) * 128] for d in range(NDC)]
            wvk_sb = [wcat_sb[:, D + d * 128:D + (d + 1) * 128] for d in range(NDC)]
            mask_sb = wcat_sb[:, 2 * D:2 * D + C]
            wot_sb = consts.tile([K, D], bf16, tag="wot")
            nc.sync.dma_start(out=wot_sb, in_=wot[:, :])
            sel_sb = consts.tile([C, 1], f32, tag="sel")
            nc.sync.dma_start(out=sel_sb, in_=sel[:, :])
            ones1_sb = consts.tile([1, 1], bf16, tag="ones1")
            nc.vector.memset(ones1_sb, 1.0)

            # running state [K, K+1]: cols 0:K = S[k, m], col K = k_sum.
            # One psum accumulation group spanning prefix + main chunks.
            state_ps = state_pool.tile([K, 1 + K], f32)

            def project_vk(psum_pool, xg, sl, tag_sfx=""):
                """[V | K_raw] token-major for one chunk; returns vko
                sbuf tile [C, 2K+1] laid out [V | ones | elu(K)+1]."""
                pp = psum_pool.tile([C, 2 * K], f32, tag="p2")
                for d in range(NDC):
                    nc.tensor.matmul(
                        pp, xg[d][:, sl], wvk_sb[d],
                        start=(d == 0), stop=(d == NDC - 1),
                    )
                vko = vko_pool.tile([C, 2 * K + 1], bf16, tag="vko")
                nc.vector.memset(vko[:, K:K + 1], 1.0)
                nc.vector.tensor_copy(vko[:, 0:K], pp[:, 0:K])
                # elu(k)+1 = exp(min(k,0)) + relu(k)
                u1 = tmp_pool.tile([C, K], f32, tag="u1")
                u2 = tmp_pool.tile([C, K], f32, tag="u2")
                nc.vector.tensor_scalar_min(u1, pp[:, K:2 * K], 0.0)
                nc.vector.tensor_scalar_max(u2, pp[:, K:2 * K], 0.0)
                eu = tmp_pool.tile([C, K], f32, tag="eu")
                nc.scalar.activation(eu, u1, AF.Exp)
                nc.vector.tensor_add(vko[:, K + 1:2 * K + 1], eu, u2)
                return vko

            # ---------------- prefix phase ----------------
            for g in range(NGRP):
                xg = []
                for d in range(NDC):
                    t = xp_pool.tile([128, G], bf16, tag=f"xp{d}")
                    nc.sync.dma_start(
                        out=t,
                        in_=xtp[d * 128:(d + 1) * 128, g * G:(g + 1) * G],
                    )
                    xg.append(t)
                for c4 in range(G // C):
                    ci = g * (G // C) + c4
                    sl = slice(c4 * C, (c4 + 1) * C)
                    vko = project_vk(p2_pool, xg, sl)
                    # sel-masked [V | ones] so first-half cores add zero
                    vks = vko_pool.tile([C, K + 1], bf16, tag="vks")
                    nc.vector.tensor_scalar_mul(vks, vko[:, 0:K + 1], sel_sb)
                    nc.tensor.matmul(
                        state_ps, vko[:, K + 1:2 * K + 1], vks,
                        start=(ci == 0), stop=False,
                        skip_group_check=True,
                    )

            # sbuf copy of the running state used as matmul lhsT
            ks_sb = small.tile([K, 1 + K], bf16, tag="ks")
            nc.scalar.copy(ks_sb, state_ps)

            # ---------------- main phase ----------------
            for g in range(NGRP):
                xg = []
                for d in range(NDC):
                    t = xm_pool.tile([128, G], bf16, tag=f"xm{d}")
                    nc.sync.dma_start(
                        out=t,
                        in_=xtm[d * 128:(d + 1) * 128, g * G:(g + 1) * G],
                    )
                    xg.append(t)
                # P1: [Q^T ; K^T] feature-major for the whole group
                p1g = p1_pool.tile([2 * K, G], f32, tag="p1")
                for d in range(NDC):
                    nc.tensor.matmul(
                        p1g, wqk_sb[d], xg[d],
                        start=(d == 0), stop=(d == NDC - 1),
                    )
                for c4 in range(G // C):
                    ci = g * (G // C) + c4
                    sl = slice(c4 * C, (c4 + 1) * C)
                    # ---- elu+1 on Q^T and K^T ----
                    t1 = tmp_pool.tile([2 * K, C], f32, tag="t1")
                    t2 = tmp_pool.tile([2 * K, C], f32, tag="t2")
                    nc.vector.tensor_scalar_min(t1, p1g[:, sl], 0.0)
                    nc.vector.tensor_scalar_max(t2, p1g[:, sl], 0.0)
                    e1 = tmp_pool.tile([2 * K, C], f32, tag="e1")
                    nc.scalar.activation(e1, t1, AF.Exp)
                    qT = qk_pool.tile([K, C], bf16, tag="qT")
                    kT = qk_pool.tile([K, C], bf16, tag="kT")
                    nc.vector.tensor_add(qT, e1[0:K, :], t2[0:K, :])
                    nc.vector.tensor_add(kT, e1[K:2 * K, :], t2[K:2 * K, :])
                    # ---- P2: [V | K] token-major ----
                    vko = project_vk(p2_pool, xg, sl)
                    # ---- intra-chunk scores A^T[t, s], masked ----
                    at = atnd_pool.tile([C, C], f32, tag="atnd")
                    nc.tensor.matmul(at, kT, qT, start=True, stop=True)
                    atm = tmp_pool.tile([C, C], bf16, tag="atm")
                    nc.vector.tensor_tensor(
                        atm, at, mask_sb, mybir.AluOpType.mult
                    )
                    # ---- [num^T ; den] = lhsT-packed pair of matmuls ----
                    nd = atnd_pool.tile([1 + K, C], f32, tag="atnd")
                    nc.tensor.matmul(
                        nd, vko[:, 0:K + 1], atm, start=True, stop=False
                    )
                    nc.tensor.matmul(
                        nd, ks_sb, qT, start=False, stop=True
                    )
                    # ---- state update, then refresh ks_sb ----
                    nc.tensor.matmul(
                        state_ps, vko[:, K + 1:2 * K + 1], vko[:, 0:K + 1],
                        start=False, stop=(ci == NCHUNK - 1),
                        skip_group_check=True,
                    )
                    nc.scalar.copy(ks_sb, state_ps)
                    # ---- reciprocal of den (transpose via 1-row matmul) --
                    den_b = small.tile([1, C], bf16, tag="den")
                    nc.vector.tensor_scalar_add(den_b, nd[K:K + 1, :], 1e-6)
                    dtp = atnd_pool.tile([C, 1], f32, tag="atnd")
                    nc.tensor.matmul(dtp, den_b, ones1_sb, start=True, stop=True)
                    recip = small.tile([C, 1], f32, tag="recip")
                    nc.vector.reciprocal(recip, dtp)
                    # ---- output projection, divide on eviction ----
                    attn = qk_pool.tile([K, C], bf16, tag="attn")
                    nc.vector.tensor_copy(attn, nd[0:K, :])
                    ost = ostage_pool.tile([C, D], f32, tag="ost")
                    for h2 in range(2):
                        op = atnd_pool.tile([C, D // 2], f32, tag="atnd")
                        nc.tensor.matmul(
                            op, attn, wot_sb[:, h2 * 512:(h2 + 1) * 512],
                            start=True, stop=True,
                        )
                        nc.scalar.activation(
                            ost[:, h2 * 512:(h2 + 1) * 512], op,
                            AF.Copy, scale=recip,
                        )
                    nc.sync.dma_start(
                        out=out[ci * C:(ci + 1) * C, :], in_=ost
                    )

    nc.compile()
    # post-lowering check: walrus MM descriptor fits 1 wait only
    worst = []
    for fn in nc.m.functions:
        for blk in fn.blocks:
            for inst in blk.instructions:
                n = len(inst.sync_info.on_wait) if inst.sync_info else 0
                if n > 1 and type(inst).__name__ == "InstMatmult":
                    worst.append((inst.name, n))
    if worst:
        raise RuntimeError(f"matmuls with >1 wait after lowering: {worst}")
    return nc


def _prep_inputs(x, Wq, Wk, Wv, Wo):
    import ml_dtypes

    bf16 = ml_dtypes.bfloat16
    wqk = np.concatenate([Wq.T, Wk.T], axis=1)                # [D, 2K]
    wvk = np.concatenate([Wv.T, Wk.T], axis=1)                # [D, 2K]
    mask = np.triu(np.ones((C, C), np.float32))               # keep t <= s
    # packed PE-side consts: [128, 2D + C] = wqk chunks | wvk chunks | mask
    wcat = np.concatenate(
        [wqk[d * 128:(d + 1) * 128, :] for d in range(NDC)]
        + [wvk[d * 128:(d + 1) * 128, :] for d in range(NDC)]
        + [mask],
        axis=1,
    ).astype(bf16)
    wot = np.ascontiguousarray(Wo.T).astype(bf16)             # [K, D]
    zeros_x = np.zeros((D, L), dtype=bf16)
    in_maps = []
    for core in range(8):
        b, h = core // 2, core % 2
        xb = np.ascontiguousarray(x[b].astype(bf16).T)        # [D, S]
        m = {
            "xtm": np.ascontiguousarray(xb[:, h * L:(h + 1) * L]),
            "xtp": np.ascontiguousarray(xb[:, 0:L]) if h else zeros_x,
            "wcat": wcat,
            "wot": wot,
            "sel": np.full((C, 1), float(h), np.float32),
        }
        in_maps.append(m)
    return in_maps


def _run(inputs, trace=False):
    from concourse.bass_utils import run_bass_kernel_spmd

    if "nc" not in _cache:
        _cache["nc"] = _build_nc()
    nc = _cache["nc"]
    in_maps = _prep_inputs(
        np.asarray(inputs["x"], np.float32),
        np.asarray(inputs["Wq"], np.float32),
        np.asarray(inputs["Wk"], np.float32),
        np.asarray(inputs["Wv"], np.float32),
        np.asarray(inputs["Wo"], np.float32),
    )
    res = run_bass_kernel_spmd(nc, in_maps, list(range(8)), trace=trace)
    out = np.empty((B, S, D), np.float32)
    for core in range(8):
        b, h = core // 2, core % 2
        out[b, h * L:(h + 1) * L, :] = res.results[core]["out"]
    return out, res


def kernel(**inputs) -> np.ndarray:
    out, _ = _run(inputs, trace=False)
    return out
